# revision 49
# baseline (speedup 1.0000x reference)
"""Causal multi-head attention with RoPE on 8 Trainium2 NeuronCores.

Sharding: core = (batch b, head-group hg): b = core//4, hg = core%4.
Each core computes 4 heads of one batch element end-to-end (QKV projection,
RoPE, causal softmax attention, output-projection partial) and the host sums
the 4 per-head-group partials per batch (the "all-reduce" of the O-proj).

v6 changes vs v2 (216us):
  - even heads (pb=0) normalize straight into oT_sb (half the place-DMAs).
  - input DMAs are issued as single descriptors per tensor (xt chunk,
    wq/wk/wv) instead of per-128-row slices: ~50 fewer queue issues.
  - chunk emission interleaves Q/K per c-half so the DVE rope chain is
    not serial at startup.
  - tail: the last head's normalization chain is covered by reserving all
    of oproj(2) plus oproj(3) for the post-attention tail.
  - cos/sin tables loaded full-size; warmup trimmed 40->28 matmuls.
Kept on the PE: causal mask (additive -1e5 matmul; a DVE multiply
re-serializes the score->exp->AV chain) and the rank-1 reciprocal
broadcast (gpsimd hops head-of-line block the norm DMA chain).
Kept off gpsimd: rope compute (its in-order queue must stay DMA-only).
Score-path and P*V matmuls run in bf16.
"""

import numpy as np

_B, _L, _D, _H, _HD = 2, 2048, 1024, 16, 64
_HPG = 4              # heads per group (per core)
_EG = _HPG * _HD      # 256
_NCORES = 8
_THETA = 10000.0
_QC = 512             # q-chunk width
_NQC = _L // _QC      # 4
_GK = 2               # k-tiles (128) per exp group
_NKC = _D // 128      # 8 contraction chunks for projections
_LC = 512             # l-chunk
_NWARM = 34           # PE warmup matmuls (256-col each)

_CACHE = {}


def _build_nc():
    from contextlib import ExitStack

    import concourse.mybir as mybir
    import concourse.tile as tile
    from concourse import bacc

    f32 = mybir.dt.float32
    f32r = mybir.dt.float32r
    bf16 = mybir.dt.bfloat16
    EXP = mybir.ActivationFunctionType.Exp

    nc = bacc.Bacc("TRN2", target_bir_lowering=False, debug=False,
                   enable_asserts=False)
    xT = nc.dram_tensor("xT", [_D, _L], bf16, kind="ExternalInput")
    wq = nc.dram_tensor("wq", [_D, _EG], bf16, kind="ExternalInput")
    wk = nc.dram_tensor("wk", [_D, _EG], bf16, kind="ExternalInput")
    wv = nc.dram_tensor("wv", [_D, _EG], bf16, kind="ExternalInput")
    wo = nc.dram_tensor("wo", [_EG, _D], bf16, kind="ExternalInput")
    cs2 = nc.dram_tensor("cs2", [128, _L], bf16, kind="ExternalInput")
    sn2 = nc.dram_tensor("sn2", [128, _L], bf16, kind="ExternalInput")
    perm = nc.dram_tensor("perm", [128, 128], bf16, kind="ExternalInput")
    maskM = nc.dram_tensor("maskM", [128, 128], bf16, kind="ExternalInput")
    id128 = nc.dram_tensor("id128", [128, 128], bf16, kind="ExternalInput")
    y = nc.dram_tensor("y", [_L, _D], f32, kind="ExternalOutput")

    with tile.TileContext(nc) as tc, ExitStack() as ctx:
        persist = ctx.enter_context(tc.tile_pool(name="persist", bufs=1))
        qT_sb = persist.tile([128, 2, _L], bf16)
        kT_sb = persist.tile([128, 2, _L], bf16)
        v_sb = persist.tile([128, _L // 128, _HPG, _HD + 4], bf16)
        oT_sb = persist.tile([128, 2, _L], bf16)
        wo_sb = persist.tile([128, 2, _D], bf16)
        wq_sb = persist.tile([128, _NKC, _EG], bf16)
        wk_sb = persist.tile([128, _NKC, _EG], bf16)
        wv_sb = persist.tile([128, _NKC, _EG], bf16)
        cs_sb = persist.tile([128, _L], bf16)
        sn_sb = persist.tile([128, _L], bf16)
        perm_sb = persist.tile([128, 128], bf16)
        mask_sb = persist.tile([128, 128], bf16)
        id_sb = persist.tile([128, 128], bf16)
        ones_sb = persist.tile([65, 64], f32r)
        warm_sb = persist.tile([128, 256], bf16)

        xtp = ctx.enter_context(tc.tile_pool(name="xtp", bufs=4))
        rtmp = ctx.enter_context(tc.tile_pool(name="rtmp", bufs=3))
        ptp = ctx.enter_context(tc.tile_pool(name="ptp", bufs=4))
        nrm = ctx.enter_context(tc.tile_pool(name="nrm", bufs=3))
        otcp = ctx.enter_context(tc.tile_pool(name="otc", bufs=2))
        # PSUM budget (8 banks): sps 2x2 + ops 2x1 + scr 2x1
        sps = ctx.enter_context(tc.tile_pool(name="sps", bufs=2, space="PSUM"))
        ops = ctx.enter_context(tc.tile_pool(name="ops", bufs=2, space="PSUM"))
        scr = ctx.enter_context(tc.tile_pool(name="scr", bufs=2, space="PSUM"))

        # --- warmup: memsets + dummy exp (forces ACT table load) + PE
        # matmul chain so the HAM clock is at 8/8 when real work lands ---
        nc.vector.memset(warm_sb.bitcast(mybir.dt.uint16), 0)
        nc.vector.memset(ones_sb.bitcast(f32), 1.0)
        nc.vector.memset(v_sb[:, :, :, _HD].bitcast(mybir.dt.uint16), 0x3F80)
        wexp = ptp.tile([1, 16], bf16, tag="pt", name="wexp")
        nc.scalar.activation(wexp, warm_sb[0:1, 0:16], EXP, scale=0.125)
        # one accumulation group: back-to-back matmuls with no semaphore
        # round-trips between them, so the HAM busy-window fills
        wp = scr.tile([128, 256], f32, tag="scr", name="warm")
        for i in range(_NWARM):
            nc.tensor.matmul(wp, warm_sb[:, 0:128], warm_sb,
                             start=(i == 0), stop=(i == _NWARM - 1))

        # --- input loads: one DMA per tensor, split across three queues in
        # first-use order ---
        wq_r = wq.rearrange("(c p) e -> p c e", p=128)
        wk_r = wk.rearrange("(c p) e -> p c e", p=128)
        wv_r = wv.rearrange("(c p) e -> p c e", p=128)
        xT_r = xT.rearrange("(c p) l -> p c l", p=128)
        # per-tensor transfers are split so they spread across hardware DMA
        # queues (a single merged DMA serializes on one queue at ~22GB/s)
        #   sync (SP):     xT chunks, y stores
        #   scalar (ACT):  wq, wv, wo
        #   gpsimd (Pool): xt0 upper half, perm/mask/id tables, wk, cos/sin
        xts = {}

        def load_xt(lc):
            xt = xtp.tile([128, _NKC, _LC], bf16, tag="xt", name=f"xt{lc}")
            for kc in range(_NKC):
                nc.sync.dma_start(out=xt[:, kc, :],
                                  in_=xT_r[:, kc, lc * _LC:(lc + 1) * _LC])
            xts[lc] = xt

        load_xt(0)
        nc.gpsimd.dma_start(out=perm_sb, in_=perm[:, :])
        nc.gpsimd.dma_start(out=mask_sb, in_=maskM[:, :])
        nc.gpsimd.dma_start(out=id_sb, in_=id128[:, :])
        for kc in range(_NKC):
            nc.scalar.dma_start(out=wq_sb[:, kc, :], in_=wq_r[:, kc, :])
            nc.gpsimd.dma_start(out=wk_sb[:, kc, :], in_=wk_r[:, kc, :])
        for q in range(4):
            qsl = slice(q * 512, (q + 1) * 512)
            nc.gpsimd.dma_start(out=cs_sb[:, qsl], in_=cs2[:, qsl])
            nc.gpsimd.dma_start(out=sn_sb[:, qsl], in_=sn2[:, qsl])
        for kc in range(_NKC):
            nc.scalar.dma_start(out=wv_sb[:, kc, :], in_=wv_r[:, kc, :])
        nc.scalar.dma_start(out=wo_sb,
                            in_=wo.rearrange("(c p) d -> p c d", p=128))
        for lc in range(1, 4):
            load_xt(lc)

        # --- work thunks ---
        def proj_qk_thunk(lc, w_sb, dst, c):
            def t():
                ls = slice(lc * _LC, (lc + 1) * _LC)
                xt = xts[lc]
                ps = scr.tile([128, _LC], f32, tag="scr",
                              name=f"ps{lc}_{c}")
                for kc in range(_NKC):
                    nc.tensor.matmul(
                        ps, w_sb[:, kc, c * 128:(c + 1) * 128],
                        xt[:, kc, :],
                        start=(kc == 0), stop=(kc == _NKC - 1))
                nc.vector.tensor_copy(dst[:, c, ls], ps)
            return t

        def rope_thunk(lc, dst, c, pool_mul=False):
            def t():
                ls = slice(lc * _LC, (lc + 1) * _LC)
                rp = scr.tile([128, _LC], f32, tag="scr",
                              name=f"rp{lc}_{c}")
                nc.tensor.matmul(rp, perm_sb[:, :], dst[:, c, ls],
                                 start=True, stop=True)
                tmp = rtmp.tile([128, _LC], bf16, tag="rt")
                nc.vector.tensor_mul(tmp, rp, sn_sb[:, ls])
                # chunk-0 K-rope runs its SBUF-only ops on gpsimd (idle at
                # startup once the DMA issues drain); elsewhere gpsimd must
                # stay DMA-only or it head-of-line blocks the norm dances
                eng = nc.gpsimd if pool_mul else nc.vector
                eng.tensor_mul(dst[:, c, ls], dst[:, c, ls], cs_sb[:, ls])
                eng.tensor_add(dst[:, c, ls], dst[:, c, ls], tmp)
            return t

        def proj_v_thunk(lc, j):
            def t():
                xt = xts[lc]
                lt = lc * (_LC // 128) + j
                pv = scr.tile([128, _EG], f32, tag="scr", name=f"pv{lt}")
                for kc in range(_NKC):
                    nc.tensor.matmul(
                        pv, xt[:, kc, j * 128:(j + 1) * 128],
                        wv_sb[:, kc, :],
                        start=(kc == 0), stop=(kc == _NKC - 1))
                nc.vector.tensor_copy(
                    v_sb[:, lt, :, :_HD],
                    pv.rearrange("p (h e) -> p h e", h=_HPG))
            return t

        def make_norm(qc, qs, h, otc, drow, row=64):
            """normalize head h of chunk qc: PE rank-1 broadcast of the
            reciprocal row, then numerators times it; even heads (pb=0)
            write oT_sb lane-aligned, odd heads stage + place-DMA"""
            def t():
                c, pb = h // 2, 64 * (h % 2)
                # rank-1 broadcast: ones[1,64].T @ recip_row -> [64, 512]
                bc = scr.tile([128, _QC], f32, tag="scr",
                              name=f"bc{qc}_{h}")
                nc.tensor.matmul(
                    bc[0:64, :], ones_sb[row:row + 1, :],
                    drow[row:row + 1, :],
                    start=True, stop=True)
                if pb == 0:
                    nc.vector.tensor_mul(oT_sb[0:64, c, qs],
                                         otc[0:64, :], bc[0:64, :])
                else:
                    otn = otcp.tile([64, _QC], bf16, tag="otn", bufs=3,
                                    name=f"otn{qc}_{h}")
                    nc.vector.tensor_mul(otn, otc[0:64, :], bc[0:64, :])
                    # partition-base shift (0 -> 64) needs a DMA
                    nc.gpsimd.dma_start(out=oT_sb[pb:pb + 64, c, qs],
                                        in_=otn)
            return t

        def oproj_thunks(qc, copy_alt=False):
            """output projection for chunk qc: 8 thunks (l-tile, n-half).
            copy_alt alternates the psum->sbuf copies between DVE and ACT
            (tail only - mid-stream ACT is exp-saturated): a pure-DVE copy
            chain throttles the psum bank recycling to ~1.3us per l-tile"""
            obs = {}

            def mk(j, n):
                def t():
                    lt = qc * (_QC // 128) + j
                    if n == 0:
                        obs[j] = otcp.tile([128, _D], f32, tag="ob",
                                           bufs=4, name=f"ob{qc}_{j}")
                    op = scr.tile([128, 512], f32, tag="scr",
                                  name=f"op{qc}_{j}_{n}")
                    for cc in range(2):
                        nc.tensor.matmul(
                            op, oT_sb[:, cc, lt * 128:(lt + 1) * 128],
                            wo_sb[:, cc, n * 512:(n + 1) * 512],
                            start=(cc == 0), stop=(cc == 1))
                    dst = obs[j][:, n * 512:(n + 1) * 512]
                    if copy_alt and n == 1:
                        nc.scalar.copy(dst, op)
                    else:
                        nc.vector.tensor_copy(dst, op)
                    # store each half as its own DMA: starts earlier and
                    # spreads the transfer over two hardware queues
                    nc.sync.dma_start(
                        out=y[lt * 128:(lt + 1) * 128,
                              n * 512:(n + 1) * 512], in_=dst)
                return t
            return [mk(j, n) for j in range(_QC // 128) for n in range(2)]

        # --- filler machinery ---
        state = {"fillers": [], "fi": 0}

        def pop_filler(n=1):
            for _ in range(n):
                if state["fi"] < len(state["fillers"]):
                    state["fillers"][state["fi"]]()
                    state["fi"] += 1

        def drain_fillers():
            pop_filler(len(state["fillers"]) - state["fi"])

        def chunk_thunks(lc):
            """projection+rope+v thunks for l-chunk lc, in dependency-
            friendly interleaved order (Q/K per c-half, then V)"""
            fl = []
            for c in range(2):
                fl.append(proj_qk_thunk(lc, wq_sb, qT_sb, c))
                fl.append(proj_qk_thunk(lc, wk_sb, kT_sb, c))
                fl.append(rope_thunk(lc, qT_sb, c))
                fl.append(rope_thunk(lc, kT_sb, c, pool_mul=(lc == 0)))
            for j in range(_LC // 128):
                fl.append(proj_v_thunk(lc, j))
            return fl

        # chunk 0 emitted directly
        for t in chunk_thunks(0):
            t()

        pend_norm = [None]
        for qc in range(_NQC):
            drain_fillers()
            # build filler list: next-chunk projections + output projection
            # deferred two chunks
            fl = []
            if qc + 1 < _NQC:
                fl.extend(chunk_thunks(qc + 1))
            if qc == 2:
                fl.extend(oproj_thunks(0))
            elif qc == 3:
                op1 = oproj_thunks(1)
                fl.extend(op1[:3])
                tail_op1 = op1[3:]
            state["fillers"] = fl
            state["fi"] = 0

            # ---- attention for q-chunk qc ----
            q0 = qc * _QC
            qs = slice(q0, q0 + _QC)
            nkt = (qc + 1) * (_QC // 128)
            ngr = (nkt + _GK - 1) // _GK

            def flush_pend(pend, qc=qc, qs=qs, nkt=nkt):
                """emit the AV matmuls of a pending exp'd group; when it is
                the head's last group, also emit the reciprocal/broadcast
                chain and the previous head's (now-ready) normalize."""
                if pend is None:
                    return
                h, kts, pt, ot = pend["h"], pend["kts"], pend["pt"], \
                    pend["ot"]
                for i, kt in enumerate(kts):
                    lo = max(kt - qc * (_QC // 128), 0) * 128
                    nc.tensor.matmul(
                        ot[:, lo:], v_sb[:, kt, h, :_HD + 1],
                        pt[:, i * _QC + lo:(i + 1) * _QC],
                        start=(kt == 0), stop=(kt == nkt - 1),
                        skip_group_check=True)
                if kts[-1] != nkt - 1:
                    return
                # head end: copy numerator+denominator rows to SBUF (frees
                # the psum bank), transpose the denominator row to [128,4]
                # so the reciprocal is partition-parallel (a [1,512] DVE
                # reciprocal measures ~6.5ns/element serial), restore to
                # a row for the PE broadcast matmul
                otc = otcp.tile([_HD + 1, _QC], f32, tag="otc", bufs=5,
                                name=f"otc{qc}_{h}")
                nc.scalar.copy(otc, ot[:, :])
                dsb = nrm.tile([128, 4], f32, tag="dsb")
                nc.gpsimd.dma_start(out=dsb, in_=otc[64:65, :])
                drec = nrm.tile([128, 4], f32r, tag="drec")
                with nc.allow_low_precision(reason="recip feeds tf32 mm"):
                    nc.vector.reciprocal(drec, dsb)
                drow = nrm.tile([65, _QC], f32r, tag="drow", bufs=4,
                                name=f"drow{qc}_{h}")
                nc.gpsimd.dma_start(out=drow[64:65, :], in_=drec)
                if pend_norm[0] is not None:
                    pend_norm[0]()
                pend_norm[0] = make_norm(qc, qs, h, otc, drow)

            pend = None
            # last chunk ends on an even head: its normalize writes oT_sb
            # directly (no place-DMA), shortening the tail chain
            horder = (1, 3, 2, 0) if qc == _NQC - 1 else range(_HPG)
            for h in horder:
                c, pb = h // 2, 64 * (h % 2)
                ot = ops.tile([_HD + 1, _QC], f32, tag="ot")
                for g in range(ngr):
                    kts = list(range(g * _GK, min((g + 1) * _GK, nkt)))
                    sp = sps.tile([128, _GK * _QC], f32, tag="sp")
                    # q columns < dj*128 of a diagonal k-tile are entirely
                    # in the causal-masked region: skip them in scores,
                    # exp and AV (triangular decomposition)
                    for i, kt in enumerate(kts):
                        dj = kt - qc * (_QC // 128)
                        lo = max(dj, 0) * 128
                        nc.tensor.matmul(
                            sp[:, i * _QC + lo:(i + 1) * _QC],
                            kT_sb[pb:pb + 64, c, kt * 128:(kt + 1) * 128],
                            qT_sb[pb:pb + 64, c, q0 + lo:q0 + _QC],
                            start=True, stop=(dj < 0),
                            skip_group_check=True)
                    # causal mask: accumulate -1e5 upper-tri into the
                    # diagonal 128-col slice of each diagonal k-tile.
                    # Emitted after BOTH scores: back-to-back with its own
                    # score it pays a ~105ns pipeline restart for the
                    # read-after-write on just-written psum columns
                    for i, kt in enumerate(kts):
                        dj = kt - qc * (_QC // 128)
                        if dj >= 0:
                            lo = dj * 128
                            nc.tensor.matmul(
                                sp[:, i * _QC + lo:i * _QC + lo + 128],
                                mask_sb[:, :], id_sb[:, :],
                                start=False, stop=True,
                                skip_group_check=True)
                    # software pipeline: the previous group's AV runs on
                    # the PE while this group's exp runs on ACT
                    flush_pend(pend)
                    pt = ptp.tile([128, _GK * _QC], bf16, tag="pt")
                    diag = any(kt - qc * (_QC // 128) >= 0 for kt in kts)
                    if not diag:
                        na = len(kts) * _QC
                        nc.scalar.activation(pt[:, :na], sp[:, :na], EXP,
                                             scale=0.125)
                    else:
                        # ragged tile starts: exp per tile's written span
                        for i, kt in enumerate(kts):
                            lo = max(kt - qc * (_QC // 128), 0) * 128
                            nc.scalar.activation(
                                pt[:, i * _QC + lo:(i + 1) * _QC],
                                sp[:, i * _QC + lo:(i + 1) * _QC], EXP,
                                scale=0.125)
                    pend = {"h": h, "kts": kts, "pt": pt, "ot": ot}
                    pop_filler()
            flush_pend(pend)
            pend = None
        # tail: the last head's normalization (whose reciprocal-dance DMAs
        # are still in flight) is sandwiched between independent oproj(2)
        # thunks so its PE broadcast matmul never heads the idle queue
        drain_fillers()
        for t in tail_op1:
            t()
        op2 = oproj_thunks(2, copy_alt=True)
        for t in op2[:5]:
            t()
        pend_norm[0]()
        pend_norm[0] = None
        for t in op2[5:]:
            t()
        for t in oproj_thunks(3, copy_alt=True):
            t()
    nc.compile()
    return nc


def get_nc():
    if "nc" not in _CACHE:
        _CACHE["nc"] = _build_nc()
    return _CACHE["nc"]


def make_in_maps(x, token_positions, Q, K, V, O_w):
    """Host-side sharding: per-core input dict (core = b*4 + hg)."""
    import ml_dtypes
    bf16 = ml_dtypes.bfloat16
    x = np.asarray(x, dtype=np.float32)
    tp = np.asarray(token_positions)
    Q = np.asarray(Q, dtype=np.float32)
    K = np.asarray(K, dtype=np.float32)
    V = np.asarray(V, dtype=np.float32)
    O_w = np.asarray(O_w, dtype=np.float32)

    # RoPE tables, [128, L]: rows = head-local e (cos/sin repeated pairwise),
    # duplicated for the two heads per partition-tile.
    i = np.arange(_HD // 2, dtype=np.float64)
    denom = _THETA ** (2.0 * i / _HD)                      # [32]
    ang = tp.astype(np.float64)[None, :] / denom[:, None]  # [32, L]
    cs2 = np.repeat(np.cos(ang), 2, axis=0).astype(np.float32)
    sn2 = np.repeat(np.sin(ang), 2, axis=0).astype(np.float32)
    cs2 = np.concatenate([cs2, cs2], axis=0)               # [128, L]
    sn2 = np.concatenate([sn2, sn2], axis=0)

    # pairwise-rotation permutation (rot(x)[2i] = -x[2i+1], rot[2i+1] = x[2i])
    # as a stationary operand: out = permT.T @ x^T = Perm @ x^T
    p64 = np.zeros((64, 64), np.float32)
    for j in range(_HD // 2):
        p64[2 * j + 1, 2 * j] = -1.0
        p64[2 * j, 2 * j + 1] = 1.0
    permT = np.zeros((128, 128), np.float32)
    permT[0:64, 0:64] = p64
    permT[64:128, 64:128] = p64

    # causal mask as an additive stationary operand: matmul adds
    # maskM.T (-1e5 where q' < k) into the diagonal score tile
    a = np.arange(128)
    maskM = np.where(a[:, None] < a[None, :], -1.0e5, 0.0).astype(np.float32)
    id128 = np.eye(128, dtype=np.float32)

    Qr = Q.reshape(_H, _HD, _D)
    Kr = K.reshape(_H, _HD, _D)
    Vr = V.reshape(_H, _HD, _D)

    in_maps = []
    xT = [np.ascontiguousarray(x[b].T).astype(bf16) for b in range(_B)]
    for core in range(_NCORES):
        b, hg = core // 4, core % 4
        hs = slice(hg * _HPG, (hg + 1) * _HPG)
        in_maps.append({
            "xT": xT[b],
            "wq": Qr[hs].reshape(_EG, _D).T.astype(bf16),
            "wk": Kr[hs].reshape(_EG, _D).T.astype(bf16),
            "wv": Vr[hs].reshape(_EG, _D).T.astype(bf16),
            "wo": O_w[:, hg * _EG:(hg + 1) * _EG].T.astype(bf16),
            "cs2": cs2.astype(bf16), "sn2": sn2.astype(bf16),
            "perm": permT.astype(bf16),
            "maskM": maskM.astype(bf16), "id128": id128.astype(bf16),
        })
    return in_maps


def run_on_hw(in_maps, trace=False, **kw):
    from concourse.bass_utils import run_bass_kernel_spmd
    nc = get_nc()
    return run_bass_kernel_spmd(nc, in_maps, core_ids=list(range(_NCORES)),
                                trace=trace, **kw)


def kernel(x, token_positions, Q, K, V, O_w):
    in_maps = make_in_maps(x, token_positions, Q, K, V, O_w)
    res = run_on_hw(in_maps)
    out = np.zeros((_B, _L, _D), dtype=np.float32)
    for core in range(_NCORES):
        out[core // 4] += res.results[core]["y"]
    return out


# revision 50
# speedup vs baseline: 1.0119x; 1.0119x over previous
"""Causal multi-head attention with RoPE on 8 Trainium2 NeuronCores.

Sharding: core = (batch b, head-group hg): b = core//4, hg = core%4.
Each core computes 4 heads of one batch element end-to-end (QKV projection,
RoPE, causal softmax attention, output-projection partial) and the host sums
the 4 per-head-group partials per batch (the "all-reduce" of the O-proj).

v6 changes vs v2 (216us):
  - even heads (pb=0) normalize straight into oT_sb (half the place-DMAs).
  - input DMAs are issued as single descriptors per tensor (xt chunk,
    wq/wk/wv) instead of per-128-row slices: ~50 fewer queue issues.
  - chunk emission interleaves Q/K per c-half so the DVE rope chain is
    not serial at startup.
  - tail: the last head's normalization chain is covered by reserving all
    of oproj(2) plus oproj(3) for the post-attention tail.
  - cos/sin tables loaded full-size; warmup trimmed 40->28 matmuls.
Kept on the PE: causal mask (additive -1e5 matmul; a DVE multiply
re-serializes the score->exp->AV chain) and the rank-1 reciprocal
broadcast (gpsimd hops head-of-line block the norm DMA chain).
Kept off gpsimd: rope compute (its in-order queue must stay DMA-only).
Score-path and P*V matmuls run in bf16.
"""

import numpy as np

_B, _L, _D, _H, _HD = 2, 2048, 1024, 16, 64
_HPG = 4              # heads per group (per core)
_EG = _HPG * _HD      # 256
_NCORES = 8
_THETA = 10000.0
_QC = 512             # q-chunk width
_NQC = _L // _QC      # 4
_GK = 2               # k-tiles (128) per exp group
_NKC = _D // 128      # 8 contraction chunks for projections
_LC = 512             # l-chunk
_NWARM = 34           # PE warmup matmuls (256-col each)

_CACHE = {}


def _build_nc():
    from contextlib import ExitStack

    import concourse.mybir as mybir
    import concourse.tile as tile
    from concourse import bacc

    f32 = mybir.dt.float32
    f32r = mybir.dt.float32r
    bf16 = mybir.dt.bfloat16
    EXP = mybir.ActivationFunctionType.Exp

    nc = bacc.Bacc("TRN2", target_bir_lowering=False, debug=False,
                   enable_asserts=False)
    xT = nc.dram_tensor("xT", [_D, _L], bf16, kind="ExternalInput")
    wq = nc.dram_tensor("wq", [_D, _EG], bf16, kind="ExternalInput")
    wk = nc.dram_tensor("wk", [_D, _EG], bf16, kind="ExternalInput")
    wv = nc.dram_tensor("wv", [_D, _EG], bf16, kind="ExternalInput")
    wo = nc.dram_tensor("wo", [_EG, _D], bf16, kind="ExternalInput")
    cs2 = nc.dram_tensor("cs2", [128, _L], bf16, kind="ExternalInput")
    sn2 = nc.dram_tensor("sn2", [128, _L], bf16, kind="ExternalInput")
    perm = nc.dram_tensor("perm", [128, 128], bf16, kind="ExternalInput")
    maskM = nc.dram_tensor("maskM", [128, 128], bf16, kind="ExternalInput")
    id128 = nc.dram_tensor("id128", [128, 128], bf16, kind="ExternalInput")
    y = nc.dram_tensor("y", [_L, _D], f32, kind="ExternalOutput")

    with tile.TileContext(nc) as tc, ExitStack() as ctx:
        persist = ctx.enter_context(tc.tile_pool(name="persist", bufs=1))
        qT_sb = persist.tile([128, 2, _L], bf16)
        kT_sb = persist.tile([128, 2, _L], bf16)
        v_sb = persist.tile([128, _L // 128, _HPG, _HD + 4], bf16)
        oT_sb = persist.tile([128, 2, _L], bf16)
        wo_sb = persist.tile([128, 2, _D], bf16)
        wq_sb = persist.tile([128, _NKC, _EG], bf16)
        wk_sb = persist.tile([128, _NKC, _EG], bf16)
        wv_sb = persist.tile([128, _NKC, _EG], bf16)
        cs_sb = persist.tile([128, _L], bf16)
        sn_sb = persist.tile([128, _L], bf16)
        perm_sb = persist.tile([128, 128], bf16)
        mask_sb = persist.tile([128, 128], bf16)
        id_sb = persist.tile([128, 128], bf16)
        ones_sb = persist.tile([65, 64], f32r)
        warm_sb = persist.tile([128, 256], bf16)

        xtp = ctx.enter_context(tc.tile_pool(name="xtp", bufs=4))
        rtmp = ctx.enter_context(tc.tile_pool(name="rtmp", bufs=3))
        ptp = ctx.enter_context(tc.tile_pool(name="ptp", bufs=4))
        nrm = ctx.enter_context(tc.tile_pool(name="nrm", bufs=3))
        otcp = ctx.enter_context(tc.tile_pool(name="otc", bufs=2))
        # PSUM budget (8 banks): sps 2x2 + ops 2x1 + scr 2x1
        sps = ctx.enter_context(tc.tile_pool(name="sps", bufs=2, space="PSUM"))
        ops = ctx.enter_context(tc.tile_pool(name="ops", bufs=2, space="PSUM"))
        scr = ctx.enter_context(tc.tile_pool(name="scr", bufs=2, space="PSUM"))

        # --- warmup: memsets + dummy exp (forces ACT table load) + PE
        # matmul chain so the HAM clock is at 8/8 when real work lands ---
        nc.vector.memset(warm_sb.bitcast(mybir.dt.uint16), 0)
        nc.vector.memset(ones_sb.bitcast(f32), 1.0)
        nc.vector.memset(v_sb[:, :, :, _HD].bitcast(mybir.dt.uint16), 0x3F80)
        wexp = ptp.tile([1, 16], bf16, tag="pt", name="wexp")
        nc.scalar.activation(wexp, warm_sb[0:1, 0:16], EXP, scale=0.125)
        # one accumulation group: back-to-back matmuls with no semaphore
        # round-trips between them, so the HAM busy-window fills
        wp = scr.tile([128, 256], f32, tag="scr", name="warm")
        for i in range(_NWARM):
            nc.tensor.matmul(wp, warm_sb[:, 0:128], warm_sb,
                             start=(i == 0), stop=(i == _NWARM - 1))

        # --- input loads: one DMA per tensor, split across three queues in
        # first-use order ---
        wq_r = wq.rearrange("(c p) e -> p c e", p=128)
        wk_r = wk.rearrange("(c p) e -> p c e", p=128)
        wv_r = wv.rearrange("(c p) e -> p c e", p=128)
        xT_r = xT.rearrange("(c p) l -> p c l", p=128)
        # per-tensor transfers are split so they spread across hardware DMA
        # queues (a single merged DMA serializes on one queue at ~22GB/s)
        #   sync (SP):     xT chunks, y stores
        #   scalar (ACT):  wq, wv, wo
        #   gpsimd (Pool): xt0 upper half, perm/mask/id tables, wk, cos/sin
        xts = {}

        def load_xt(lc):
            xt = xtp.tile([128, _NKC, _LC], bf16, tag="xt", name=f"xt{lc}")
            for kc in range(_NKC):
                nc.sync.dma_start(out=xt[:, kc, :],
                                  in_=xT_r[:, kc, lc * _LC:(lc + 1) * _LC])
            xts[lc] = xt

        load_xt(0)
        nc.gpsimd.dma_start(out=perm_sb, in_=perm[:, :])
        nc.gpsimd.dma_start(out=mask_sb, in_=maskM[:, :])
        nc.gpsimd.dma_start(out=id_sb, in_=id128[:, :])
        for kc in range(_NKC):
            nc.scalar.dma_start(out=wq_sb[:, kc, :], in_=wq_r[:, kc, :])
            nc.gpsimd.dma_start(out=wk_sb[:, kc, :], in_=wk_r[:, kc, :])
        for q in range(4):
            qsl = slice(q * 512, (q + 1) * 512)
            nc.gpsimd.dma_start(out=cs_sb[:, qsl], in_=cs2[:, qsl])
            nc.gpsimd.dma_start(out=sn_sb[:, qsl], in_=sn2[:, qsl])
        for kc in range(_NKC):
            nc.scalar.dma_start(out=wv_sb[:, kc, :], in_=wv_r[:, kc, :])
        nc.scalar.dma_start(out=wo_sb,
                            in_=wo.rearrange("(c p) d -> p c d", p=128))
        for lc in range(1, 4):
            load_xt(lc)

        # --- work thunks ---
        def proj_qk_thunk(lc, w_sb, dst, c):
            def t():
                ls = slice(lc * _LC, (lc + 1) * _LC)
                xt = xts[lc]
                ps = scr.tile([128, _LC], f32, tag="scr",
                              name=f"ps{lc}_{c}")
                for kc in range(_NKC):
                    nc.tensor.matmul(
                        ps, w_sb[:, kc, c * 128:(c + 1) * 128],
                        xt[:, kc, :],
                        start=(kc == 0), stop=(kc == _NKC - 1))
                nc.vector.tensor_copy(dst[:, c, ls], ps)
            return t

        def rope_thunk(lc, dst, c, pool_mul=False):
            def t():
                ls = slice(lc * _LC, (lc + 1) * _LC)
                rp = scr.tile([128, _LC], f32, tag="scr",
                              name=f"rp{lc}_{c}")
                nc.tensor.matmul(rp, perm_sb[:, :], dst[:, c, ls],
                                 start=True, stop=True)
                tmp = rtmp.tile([128, _LC], bf16, tag="rt")
                nc.vector.tensor_mul(tmp, rp, sn_sb[:, ls])
                # chunk-0 K-rope runs its SBUF-only ops on gpsimd (idle at
                # startup once the DMA issues drain); elsewhere gpsimd must
                # stay DMA-only or it head-of-line blocks the norm dances
                eng = nc.gpsimd if pool_mul else nc.vector
                eng.tensor_mul(dst[:, c, ls], dst[:, c, ls], cs_sb[:, ls])
                eng.tensor_add(dst[:, c, ls], dst[:, c, ls], tmp)
            return t

        def proj_v_thunk(lc, j):
            def t():
                xt = xts[lc]
                lt = lc * (_LC // 128) + j
                pv = scr.tile([128, _EG], f32, tag="scr", name=f"pv{lt}")
                for kc in range(_NKC):
                    nc.tensor.matmul(
                        pv, xt[:, kc, j * 128:(j + 1) * 128],
                        wv_sb[:, kc, :],
                        start=(kc == 0), stop=(kc == _NKC - 1))
                nc.vector.tensor_copy(
                    v_sb[:, lt, :, :_HD],
                    pv.rearrange("p (h e) -> p h e", h=_HPG))
            return t

        def make_norm(qc, qs, h, otc, drow, row=64):
            """normalize head h of chunk qc: PE rank-1 broadcast of the
            reciprocal row, then numerators times it; even heads (pb=0)
            write oT_sb lane-aligned, odd heads stage + place-DMA"""
            def t():
                c, pb = h // 2, 64 * (h % 2)
                # rank-1 broadcast: ones[1,64].T @ recip_row -> [64, 512]
                bc = scr.tile([128, _QC], f32, tag="scr",
                              name=f"bc{qc}_{h}")
                nc.tensor.matmul(
                    bc[0:64, :], ones_sb[row:row + 1, :],
                    drow[row:row + 1, :],
                    start=True, stop=True)
                if pb == 0:
                    nc.vector.tensor_mul(oT_sb[0:64, c, qs],
                                         otc[0:64, :], bc[0:64, :])
                else:
                    otn = otcp.tile([64, _QC], bf16, tag="otn", bufs=3,
                                    name=f"otn{qc}_{h}")
                    nc.vector.tensor_mul(otn, otc[0:64, :], bc[0:64, :])
                    # partition-base shift (0 -> 64) needs a DMA
                    nc.gpsimd.dma_start(out=oT_sb[pb:pb + 64, c, qs],
                                        in_=otn)
            return t

        def oproj_thunks(qc, copy_alt=False):
            """output projection for chunk qc: 8 thunks (l-tile, n-half).
            copy_alt alternates the psum->sbuf copies between DVE and ACT
            (tail only - mid-stream ACT is exp-saturated): a pure-DVE copy
            chain throttles the psum bank recycling to ~1.3us per l-tile"""
            obs = {}

            def mk(j, n):
                def t():
                    lt = qc * (_QC // 128) + j
                    if n == 0:
                        obs[j] = otcp.tile([128, _D], f32, tag="ob",
                                           bufs=4, name=f"ob{qc}_{j}")
                    op = scr.tile([128, 512], f32, tag="scr",
                                  name=f"op{qc}_{j}_{n}")
                    for cc in range(2):
                        nc.tensor.matmul(
                            op, oT_sb[:, cc, lt * 128:(lt + 1) * 128],
                            wo_sb[:, cc, n * 512:(n + 1) * 512],
                            start=(cc == 0), stop=(cc == 1))
                    dst = obs[j][:, n * 512:(n + 1) * 512]
                    if copy_alt and n == 1:
                        nc.scalar.copy(dst, op)
                    else:
                        nc.vector.tensor_copy(dst, op)
                    # store each half as its own DMA: starts earlier and
                    # spreads the transfer over two hardware queues
                    nc.sync.dma_start(
                        out=y[lt * 128:(lt + 1) * 128,
                              n * 512:(n + 1) * 512], in_=dst)
                return t
            return [mk(j, n) for j in range(_QC // 128) for n in range(2)]

        # --- filler machinery ---
        state = {"fillers": [], "fi": 0}

        def pop_filler(n=1):
            for _ in range(n):
                if state["fi"] < len(state["fillers"]):
                    state["fillers"][state["fi"]]()
                    state["fi"] += 1

        def drain_fillers():
            pop_filler(len(state["fillers"]) - state["fi"])

        def chunk_thunks(lc):
            """projection+rope+v thunks for l-chunk lc, in dependency-
            friendly interleaved order (Q/K per c-half, then V)"""
            fl = []
            for c in range(2):
                fl.append(proj_qk_thunk(lc, wq_sb, qT_sb, c))
                fl.append(proj_qk_thunk(lc, wk_sb, kT_sb, c))
                fl.append(rope_thunk(lc, qT_sb, c))
                fl.append(rope_thunk(lc, kT_sb, c, pool_mul=(lc == 0)))
            for j in range(_LC // 128):
                fl.append(proj_v_thunk(lc, j))
            return fl

        # chunk 0 emitted directly
        for t in chunk_thunks(0):
            t()

        pend_norm = [None]
        for qc in range(_NQC):
            drain_fillers()
            # build filler list: next-chunk projections + output projection
            # deferred two chunks
            fl = []
            if qc + 1 < _NQC:
                fl.extend(chunk_thunks(qc + 1))
            if qc == 2:
                fl.extend(oproj_thunks(0))
            elif qc == 3:
                op1 = oproj_thunks(1)
                fl.extend(op1)
                tail_op1 = []
            state["fillers"] = fl
            state["fi"] = 0

            # ---- attention for q-chunk qc ----
            q0 = qc * _QC
            qs = slice(q0, q0 + _QC)
            nkt = (qc + 1) * (_QC // 128)
            ngr = (nkt + _GK - 1) // _GK

            def flush_pend(pend, qc=qc, qs=qs, nkt=nkt):
                """emit the AV matmuls of a pending exp'd group; when it is
                the head's last group, also emit the reciprocal/broadcast
                chain and the previous head's (now-ready) normalize."""
                if pend is None:
                    return
                h, kts, pt, ot = pend["h"], pend["kts"], pend["pt"], \
                    pend["ot"]
                for i, kt in enumerate(kts):
                    lo = max(kt - qc * (_QC // 128), 0) * 128
                    nc.tensor.matmul(
                        ot[:, lo:], v_sb[:, kt, h, :_HD + 1],
                        pt[:, i * _QC + lo:(i + 1) * _QC],
                        start=(kt == 0), stop=(kt == nkt - 1),
                        skip_group_check=True)
                if kts[-1] != nkt - 1:
                    return
                # head end: copy numerator+denominator rows to SBUF (frees
                # the psum bank), transpose the denominator row to [128,4]
                # so the reciprocal is partition-parallel (a [1,512] DVE
                # reciprocal measures ~6.5ns/element serial), restore to
                # a row for the PE broadcast matmul
                otc = otcp.tile([_HD + 1, _QC], f32, tag="otc", bufs=5,
                                name=f"otc{qc}_{h}")
                nc.scalar.copy(otc, ot[:, :])
                dsb = nrm.tile([128, 4], f32, tag="dsb")
                nc.gpsimd.dma_start(out=dsb, in_=otc[64:65, :])
                drec = nrm.tile([128, 4], f32r, tag="drec")
                with nc.allow_low_precision(reason="recip feeds tf32 mm"):
                    nc.vector.reciprocal(drec, dsb)
                drow = nrm.tile([65, _QC], f32r, tag="drow", bufs=4,
                                name=f"drow{qc}_{h}")
                nc.gpsimd.dma_start(out=drow[64:65, :], in_=drec)
                if pend_norm[0] is not None:
                    pend_norm[0]()
                pend_norm[0] = make_norm(qc, qs, h, otc, drow)

            pend = None
            # last chunk ends on an even head: its normalize writes oT_sb
            # directly (no place-DMA), shortening the tail chain
            horder = (1, 3, 2, 0) if qc == _NQC - 1 else range(_HPG)
            for h in horder:
                c, pb = h // 2, 64 * (h % 2)
                ot = ops.tile([_HD + 1, _QC], f32, tag="ot")
                for g in range(ngr):
                    kts = list(range(g * _GK, min((g + 1) * _GK, nkt)))
                    sp = sps.tile([128, _GK * _QC], f32, tag="sp")
                    # q columns < dj*128 of a diagonal k-tile are entirely
                    # in the causal-masked region: skip them in scores,
                    # exp and AV (triangular decomposition)
                    for i, kt in enumerate(kts):
                        dj = kt - qc * (_QC // 128)
                        lo = max(dj, 0) * 128
                        nc.tensor.matmul(
                            sp[:, i * _QC + lo:(i + 1) * _QC],
                            kT_sb[pb:pb + 64, c, kt * 128:(kt + 1) * 128],
                            qT_sb[pb:pb + 64, c, q0 + lo:q0 + _QC],
                            start=True, stop=(dj < 0),
                            skip_group_check=True)
                    # causal mask: accumulate -1e5 upper-tri into the
                    # diagonal 128-col slice of each diagonal k-tile.
                    # Emitted after BOTH scores: back-to-back with its own
                    # score it pays a ~105ns pipeline restart for the
                    # read-after-write on just-written psum columns
                    for i, kt in enumerate(kts):
                        dj = kt - qc * (_QC // 128)
                        if dj >= 0:
                            lo = dj * 128
                            nc.tensor.matmul(
                                sp[:, i * _QC + lo:i * _QC + lo + 128],
                                mask_sb[:, :], id_sb[:, :],
                                start=False, stop=True,
                                skip_group_check=True)
                    # software pipeline: the previous group's AV runs on
                    # the PE while this group's exp runs on ACT
                    flush_pend(pend)
                    pt = ptp.tile([128, _GK * _QC], bf16, tag="pt")
                    diag = any(kt - qc * (_QC // 128) >= 0 for kt in kts)
                    if not diag:
                        na = len(kts) * _QC
                        nc.scalar.activation(pt[:, :na], sp[:, :na], EXP,
                                             scale=0.125)
                    else:
                        # ragged tile starts: exp per tile's written span
                        for i, kt in enumerate(kts):
                            lo = max(kt - qc * (_QC // 128), 0) * 128
                            nc.scalar.activation(
                                pt[:, i * _QC + lo:(i + 1) * _QC],
                                sp[:, i * _QC + lo:(i + 1) * _QC], EXP,
                                scale=0.125)
                    pend = {"h": h, "kts": kts, "pt": pt, "ot": ot}
                    pop_filler()
            flush_pend(pend)
            pend = None
        # tail: the last head's normalization (whose reciprocal-dance DMAs
        # are still in flight) is sandwiched between independent oproj(2)
        # thunks so its PE broadcast matmul never heads the idle queue
        drain_fillers()
        for t in tail_op1:
            t()
        op2 = oproj_thunks(2, copy_alt=True)
        for t in op2[:5]:
            t()
        pend_norm[0]()
        pend_norm[0] = None
        for t in op2[5:]:
            t()
        for t in oproj_thunks(3, copy_alt=True):
            t()
    nc.compile()
    return nc


def get_nc():
    if "nc" not in _CACHE:
        _CACHE["nc"] = _build_nc()
    return _CACHE["nc"]


def make_in_maps(x, token_positions, Q, K, V, O_w):
    """Host-side sharding: per-core input dict (core = b*4 + hg)."""
    import ml_dtypes
    bf16 = ml_dtypes.bfloat16
    x = np.asarray(x, dtype=np.float32)
    tp = np.asarray(token_positions)
    Q = np.asarray(Q, dtype=np.float32)
    K = np.asarray(K, dtype=np.float32)
    V = np.asarray(V, dtype=np.float32)
    O_w = np.asarray(O_w, dtype=np.float32)

    # RoPE tables, [128, L]: rows = head-local e (cos/sin repeated pairwise),
    # duplicated for the two heads per partition-tile.
    i = np.arange(_HD // 2, dtype=np.float64)
    denom = _THETA ** (2.0 * i / _HD)                      # [32]
    ang = tp.astype(np.float64)[None, :] / denom[:, None]  # [32, L]
    cs2 = np.repeat(np.cos(ang), 2, axis=0).astype(np.float32)
    sn2 = np.repeat(np.sin(ang), 2, axis=0).astype(np.float32)
    cs2 = np.concatenate([cs2, cs2], axis=0)               # [128, L]
    sn2 = np.concatenate([sn2, sn2], axis=0)

    # pairwise-rotation permutation (rot(x)[2i] = -x[2i+1], rot[2i+1] = x[2i])
    # as a stationary operand: out = permT.T @ x^T = Perm @ x^T
    p64 = np.zeros((64, 64), np.float32)
    for j in range(_HD // 2):
        p64[2 * j + 1, 2 * j] = -1.0
        p64[2 * j, 2 * j + 1] = 1.0
    permT = np.zeros((128, 128), np.float32)
    permT[0:64, 0:64] = p64
    permT[64:128, 64:128] = p64

    # causal mask as an additive stationary operand: matmul adds
    # maskM.T (-1e5 where q' < k) into the diagonal score tile
    a = np.arange(128)
    maskM = np.where(a[:, None] < a[None, :], -1.0e5, 0.0).astype(np.float32)
    id128 = np.eye(128, dtype=np.float32)

    Qr = Q.reshape(_H, _HD, _D)
    Kr = K.reshape(_H, _HD, _D)
    Vr = V.reshape(_H, _HD, _D)

    in_maps = []
    xT = [np.ascontiguousarray(x[b].T).astype(bf16) for b in range(_B)]
    for core in range(_NCORES):
        b, hg = core // 4, core % 4
        hs = slice(hg * _HPG, (hg + 1) * _HPG)
        in_maps.append({
            "xT": xT[b],
            "wq": Qr[hs].reshape(_EG, _D).T.astype(bf16),
            "wk": Kr[hs].reshape(_EG, _D).T.astype(bf16),
            "wv": Vr[hs].reshape(_EG, _D).T.astype(bf16),
            "wo": O_w[:, hg * _EG:(hg + 1) * _EG].T.astype(bf16),
            "cs2": cs2.astype(bf16), "sn2": sn2.astype(bf16),
            "perm": permT.astype(bf16),
            "maskM": maskM.astype(bf16), "id128": id128.astype(bf16),
        })
    return in_maps


def run_on_hw(in_maps, trace=False, **kw):
    from concourse.bass_utils import run_bass_kernel_spmd
    nc = get_nc()
    return run_bass_kernel_spmd(nc, in_maps, core_ids=list(range(_NCORES)),
                                trace=trace, **kw)


def kernel(x, token_positions, Q, K, V, O_w):
    in_maps = make_in_maps(x, token_positions, Q, K, V, O_w)
    res = run_on_hw(in_maps)
    out = np.zeros((_B, _L, _D), dtype=np.float32)
    for core in range(_NCORES):
        out[core // 4] += res.results[core]["y"]
    return out


# revision 51
# speedup vs baseline: 1.0121x; 1.0002x over previous
"""Causal multi-head attention with RoPE on 8 Trainium2 NeuronCores.

Sharding: core = (batch b, head-group hg): b = core//4, hg = core%4.
Each core computes 4 heads of one batch element end-to-end (QKV projection,
RoPE, causal softmax attention, output-projection partial) and the host sums
the 4 per-head-group partials per batch (the "all-reduce" of the O-proj).

v3 changes vs v2 (216us -> ~198us):
  - diagonal-group mask matmuls emitted after BOTH score matmuls of the
    group: back-to-back with its own score each paid a ~105ns pipeline
    restart for the read-after-write on just-written psum columns.
  - even heads (pb=0) normalize straight into oT_sb (half the place-DMAs);
    last chunk processes heads (1,3,2,0) so the tail head writes direct.
  - tail restructured: held-back oproj(2) thunks sandwich the final norm
    so its PE broadcast never heads an idle in-order queue, and the
    psum->sbuf staging copies alternate DVE/ACT (a pure-DVE copy chain
    throttles psum-bank recycling); obs staging 4-deep; y stored as two
    half-DMAs per l-tile (starts earlier, two hardware queues).
  - chunk emission interleaves Q/K per c-half; chunk-0 K-rope cos/add on
    gpsimd (idle once the DMA issues drain); cos/sin loaded full-size.
Negative results (reverted): tri-mask on DVE (re-serializes the
score->exp->AV chain), [1,512] DVE reciprocal (6.5ns/elem serial),
partition_broadcast/rope on gpsimd mid-stream (its in-order queue
head-of-line blocks the norm dance DMA issues), per-tensor merged DMAs
(serialize on one ~22GB/s hardware queue), PE is_transpose for the
reciprocal row (neuronxcc backend crash).
Score-path and P*V matmuls run in bf16.
"""

import numpy as np

_B, _L, _D, _H, _HD = 2, 2048, 1024, 16, 64
_HPG = 4              # heads per group (per core)
_EG = _HPG * _HD      # 256
_NCORES = 8
_THETA = 10000.0
_QC = 512             # q-chunk width
_NQC = _L // _QC      # 4
_GK = 2               # k-tiles (128) per exp group
_NKC = _D // 128      # 8 contraction chunks for projections
_LC = 512             # l-chunk
_NWARM = 34           # PE warmup matmuls (256-col each)

_CACHE = {}


def _build_nc():
    from contextlib import ExitStack

    import concourse.mybir as mybir
    import concourse.tile as tile
    from concourse import bacc

    f32 = mybir.dt.float32
    f32r = mybir.dt.float32r
    bf16 = mybir.dt.bfloat16
    EXP = mybir.ActivationFunctionType.Exp

    nc = bacc.Bacc("TRN2", target_bir_lowering=False, debug=False,
                   enable_asserts=False)
    xT = nc.dram_tensor("xT", [_D, _L], bf16, kind="ExternalInput")
    wq = nc.dram_tensor("wq", [_D, _EG], bf16, kind="ExternalInput")
    wk = nc.dram_tensor("wk", [_D, _EG], bf16, kind="ExternalInput")
    wv = nc.dram_tensor("wv", [_D, _EG], bf16, kind="ExternalInput")
    wo = nc.dram_tensor("wo", [_EG, _D], bf16, kind="ExternalInput")
    cs2 = nc.dram_tensor("cs2", [128, _L], bf16, kind="ExternalInput")
    sn2 = nc.dram_tensor("sn2", [128, _L], bf16, kind="ExternalInput")
    perm = nc.dram_tensor("perm", [128, 128], bf16, kind="ExternalInput")
    maskM = nc.dram_tensor("maskM", [128, 128], bf16, kind="ExternalInput")
    id128 = nc.dram_tensor("id128", [128, 128], bf16, kind="ExternalInput")
    y = nc.dram_tensor("y", [_L, _D], f32, kind="ExternalOutput")

    with tile.TileContext(nc) as tc, ExitStack() as ctx:
        persist = ctx.enter_context(tc.tile_pool(name="persist", bufs=1))
        qT_sb = persist.tile([128, 2, _L], bf16)
        kT_sb = persist.tile([128, 2, _L], bf16)
        v_sb = persist.tile([128, _L // 128, _HPG, _HD + 4], bf16)
        oT_sb = persist.tile([128, 2, _L], bf16)
        wo_sb = persist.tile([128, 2, _D], bf16)
        wq_sb = persist.tile([128, _NKC, _EG], bf16)
        wk_sb = persist.tile([128, _NKC, _EG], bf16)
        wv_sb = persist.tile([128, _NKC, _EG], bf16)
        cs_sb = persist.tile([128, _L], bf16)
        sn_sb = persist.tile([128, _L], bf16)
        perm_sb = persist.tile([128, 128], bf16)
        mask_sb = persist.tile([128, 128], bf16)
        id_sb = persist.tile([128, 128], bf16)
        ones_sb = persist.tile([65, 64], f32r)
        warm_sb = persist.tile([128, 256], bf16)

        xtp = ctx.enter_context(tc.tile_pool(name="xtp", bufs=4))
        rtmp = ctx.enter_context(tc.tile_pool(name="rtmp", bufs=3))
        ptp = ctx.enter_context(tc.tile_pool(name="ptp", bufs=4))
        nrm = ctx.enter_context(tc.tile_pool(name="nrm", bufs=3))
        otcp = ctx.enter_context(tc.tile_pool(name="otc", bufs=2))
        # PSUM budget (8 banks): sps 2x2 + ops 2x1 + scr 2x1
        sps = ctx.enter_context(tc.tile_pool(name="sps", bufs=2, space="PSUM"))
        ops = ctx.enter_context(tc.tile_pool(name="ops", bufs=2, space="PSUM"))
        scr = ctx.enter_context(tc.tile_pool(name="scr", bufs=2, space="PSUM"))

        # --- warmup: memsets + dummy exp (forces ACT table load) + PE
        # matmul chain so the HAM clock is at 8/8 when real work lands ---
        nc.vector.memset(warm_sb.bitcast(mybir.dt.uint16), 0)
        nc.vector.memset(ones_sb.bitcast(f32), 1.0)
        nc.vector.memset(v_sb[:, :, :, _HD].bitcast(mybir.dt.uint16), 0x3F80)
        wexp = ptp.tile([1, 16], bf16, tag="pt", name="wexp")
        nc.scalar.activation(wexp, warm_sb[0:1, 0:16], EXP, scale=0.125)
        # one accumulation group: back-to-back matmuls with no semaphore
        # round-trips between them, so the HAM busy-window fills
        wp = scr.tile([128, 256], f32, tag="scr", name="warm")
        for i in range(_NWARM):
            nc.tensor.matmul(wp, warm_sb[:, 0:128], warm_sb,
                             start=(i == 0), stop=(i == _NWARM - 1))

        # --- input loads: one DMA per tensor, split across three queues in
        # first-use order ---
        wq_r = wq.rearrange("(c p) e -> p c e", p=128)
        wk_r = wk.rearrange("(c p) e -> p c e", p=128)
        wv_r = wv.rearrange("(c p) e -> p c e", p=128)
        xT_r = xT.rearrange("(c p) l -> p c l", p=128)
        # per-tensor transfers are split so they spread across hardware DMA
        # queues (a single merged DMA serializes on one queue at ~22GB/s)
        #   sync (SP):     xT chunks, y stores
        #   scalar (ACT):  wq, wv, wo
        #   gpsimd (Pool): xt0 upper half, perm/mask/id tables, wk, cos/sin
        xts = {}

        def load_xt(lc):
            xt = xtp.tile([128, _NKC, _LC], bf16, tag="xt", name=f"xt{lc}")
            for kc in range(_NKC):
                nc.sync.dma_start(out=xt[:, kc, :],
                                  in_=xT_r[:, kc, lc * _LC:(lc + 1) * _LC])
            xts[lc] = xt

        load_xt(0)
        nc.gpsimd.dma_start(out=perm_sb, in_=perm[:, :])
        nc.gpsimd.dma_start(out=mask_sb, in_=maskM[:, :])
        nc.gpsimd.dma_start(out=id_sb, in_=id128[:, :])
        for kc in range(_NKC):
            nc.scalar.dma_start(out=wq_sb[:, kc, :], in_=wq_r[:, kc, :])
            nc.gpsimd.dma_start(out=wk_sb[:, kc, :], in_=wk_r[:, kc, :])
        for q in range(4):
            qsl = slice(q * 512, (q + 1) * 512)
            nc.gpsimd.dma_start(out=cs_sb[:, qsl], in_=cs2[:, qsl])
            nc.gpsimd.dma_start(out=sn_sb[:, qsl], in_=sn2[:, qsl])
        for kc in range(_NKC):
            nc.scalar.dma_start(out=wv_sb[:, kc, :], in_=wv_r[:, kc, :])
        nc.scalar.dma_start(out=wo_sb,
                            in_=wo.rearrange("(c p) d -> p c d", p=128))
        for lc in range(1, 4):
            load_xt(lc)

        # --- work thunks ---
        def proj_qk_thunk(lc, w_sb, dst, c):
            def t():
                ls = slice(lc * _LC, (lc + 1) * _LC)
                xt = xts[lc]
                ps = scr.tile([128, _LC], f32, tag="scr",
                              name=f"ps{lc}_{c}")
                for kc in range(_NKC):
                    nc.tensor.matmul(
                        ps, w_sb[:, kc, c * 128:(c + 1) * 128],
                        xt[:, kc, :],
                        start=(kc == 0), stop=(kc == _NKC - 1))
                nc.vector.tensor_copy(dst[:, c, ls], ps)
            return t

        def rope_thunk(lc, dst, c, pool_mul=False):
            def t():
                ls = slice(lc * _LC, (lc + 1) * _LC)
                rp = scr.tile([128, _LC], f32, tag="scr",
                              name=f"rp{lc}_{c}")
                nc.tensor.matmul(rp, perm_sb[:, :], dst[:, c, ls],
                                 start=True, stop=True)
                tmp = rtmp.tile([128, _LC], bf16, tag="rt")
                nc.vector.tensor_mul(tmp, rp, sn_sb[:, ls])
                # chunk-0 K-rope runs its SBUF-only ops on gpsimd (idle at
                # startup once the DMA issues drain); elsewhere gpsimd must
                # stay DMA-only or it head-of-line blocks the norm dances
                eng = nc.gpsimd if pool_mul else nc.vector
                eng.tensor_mul(dst[:, c, ls], dst[:, c, ls], cs_sb[:, ls])
                eng.tensor_add(dst[:, c, ls], dst[:, c, ls], tmp)
            return t

        def proj_v_thunk(lc, j):
            def t():
                xt = xts[lc]
                lt = lc * (_LC // 128) + j
                pv = scr.tile([128, _EG], f32, tag="scr", name=f"pv{lt}")
                for kc in range(_NKC):
                    nc.tensor.matmul(
                        pv, xt[:, kc, j * 128:(j + 1) * 128],
                        wv_sb[:, kc, :],
                        start=(kc == 0), stop=(kc == _NKC - 1))
                nc.vector.tensor_copy(
                    v_sb[:, lt, :, :_HD],
                    pv.rearrange("p (h e) -> p h e", h=_HPG))
            return t

        def make_norm(qc, qs, h, otc, drow, row=64):
            """normalize head h of chunk qc: PE rank-1 broadcast of the
            reciprocal row, then numerators times it; even heads (pb=0)
            write oT_sb lane-aligned, odd heads stage + place-DMA"""
            def t():
                c, pb = h // 2, 64 * (h % 2)
                # rank-1 broadcast: ones[1,64].T @ recip_row -> [64, 512]
                bc = scr.tile([128, _QC], f32, tag="scr",
                              name=f"bc{qc}_{h}")
                nc.tensor.matmul(
                    bc[0:64, :], ones_sb[row:row + 1, :],
                    drow[row:row + 1, :],
                    start=True, stop=True)
                if pb == 0:
                    nc.vector.tensor_mul(oT_sb[0:64, c, qs],
                                         otc[0:64, :], bc[0:64, :])
                else:
                    otn = otcp.tile([64, _QC], bf16, tag="otn", bufs=3,
                                    name=f"otn{qc}_{h}")
                    nc.vector.tensor_mul(otn, otc[0:64, :], bc[0:64, :])
                    # partition-base shift (0 -> 64) needs a DMA
                    nc.gpsimd.dma_start(out=oT_sb[pb:pb + 64, c, qs],
                                        in_=otn)
            return t

        def oproj_thunks(qc, copy_alt=False):
            """output projection for chunk qc: 8 thunks (l-tile, n-half).
            copy_alt alternates the psum->sbuf copies between DVE and ACT
            (tail only - mid-stream ACT is exp-saturated): a pure-DVE copy
            chain throttles the psum bank recycling to ~1.3us per l-tile"""
            obs = {}

            def mk(j, n):
                def t():
                    lt = qc * (_QC // 128) + j
                    if n == 0:
                        obs[j] = otcp.tile([128, _D], f32, tag="ob",
                                           bufs=4, name=f"ob{qc}_{j}")
                    op = scr.tile([128, 512], f32, tag="scr",
                                  name=f"op{qc}_{j}_{n}")
                    for cc in range(2):
                        nc.tensor.matmul(
                            op, oT_sb[:, cc, lt * 128:(lt + 1) * 128],
                            wo_sb[:, cc, n * 512:(n + 1) * 512],
                            start=(cc == 0), stop=(cc == 1))
                    dst = obs[j][:, n * 512:(n + 1) * 512]
                    if copy_alt and n == 1:
                        nc.scalar.copy(dst, op)
                    else:
                        nc.vector.tensor_copy(dst, op)
                    # store each half as its own DMA: starts earlier and
                    # spreads the transfer over two hardware queues
                    nc.sync.dma_start(
                        out=y[lt * 128:(lt + 1) * 128,
                              n * 512:(n + 1) * 512], in_=dst)
                return t
            return [mk(j, n) for j in range(_QC // 128) for n in range(2)]

        # --- filler machinery ---
        state = {"fillers": [], "fi": 0}

        def pop_filler(n=1):
            for _ in range(n):
                if state["fi"] < len(state["fillers"]):
                    state["fillers"][state["fi"]]()
                    state["fi"] += 1

        def drain_fillers():
            pop_filler(len(state["fillers"]) - state["fi"])

        def chunk_thunks(lc):
            """projection+rope+v thunks for l-chunk lc, in dependency-
            friendly interleaved order (Q/K per c-half, then V)"""
            fl = []
            for c in range(2):
                fl.append(proj_qk_thunk(lc, wq_sb, qT_sb, c))
                fl.append(proj_qk_thunk(lc, wk_sb, kT_sb, c))
                fl.append(rope_thunk(lc, qT_sb, c))
                fl.append(rope_thunk(lc, kT_sb, c, pool_mul=(lc == 0)))
            for j in range(_LC // 128):
                fl.append(proj_v_thunk(lc, j))
            return fl

        # chunk 0 emitted directly
        for t in chunk_thunks(0):
            t()

        pend_norm = [None]
        for qc in range(_NQC):
            drain_fillers()
            # build filler list: next-chunk projections + output projection
            # deferred two chunks
            fl = []
            if qc + 1 < _NQC:
                fl.extend(chunk_thunks(qc + 1))
            if qc == 2:
                fl.extend(oproj_thunks(0))
            elif qc == 3:
                op1 = oproj_thunks(1)
                fl.extend(op1)
                tail_op1 = []
            state["fillers"] = fl
            state["fi"] = 0

            # ---- attention for q-chunk qc ----
            q0 = qc * _QC
            qs = slice(q0, q0 + _QC)
            nkt = (qc + 1) * (_QC // 128)
            ngr = (nkt + _GK - 1) // _GK

            def flush_pend(pend, qc=qc, qs=qs, nkt=nkt):
                """emit the AV matmuls of a pending exp'd group; when it is
                the head's last group, also emit the reciprocal/broadcast
                chain and the previous head's (now-ready) normalize."""
                if pend is None:
                    return
                h, kts, pt, ot = pend["h"], pend["kts"], pend["pt"], \
                    pend["ot"]
                for i, kt in enumerate(kts):
                    lo = max(kt - qc * (_QC // 128), 0) * 128
                    nc.tensor.matmul(
                        ot[:, lo:], v_sb[:, kt, h, :_HD + 1],
                        pt[:, i * _QC + lo:(i + 1) * _QC],
                        start=(kt == 0), stop=(kt == nkt - 1),
                        skip_group_check=True)
                if kts[-1] != nkt - 1:
                    return
                # head end: copy numerator+denominator rows to SBUF (frees
                # the psum bank), transpose the denominator row to [128,4]
                # so the reciprocal is partition-parallel (a [1,512] DVE
                # reciprocal measures ~6.5ns/element serial), restore to
                # a row for the PE broadcast matmul
                otc = otcp.tile([_HD + 1, _QC], f32, tag="otc", bufs=5,
                                name=f"otc{qc}_{h}")
                nc.scalar.copy(otc, ot[:, :])
                dsb = nrm.tile([128, 4], f32, tag="dsb")
                nc.gpsimd.dma_start(out=dsb, in_=otc[64:65, :])
                drec = nrm.tile([128, 4], f32r, tag="drec")
                with nc.allow_low_precision(reason="recip feeds tf32 mm"):
                    nc.vector.reciprocal(drec, dsb)
                drow = nrm.tile([65, _QC], f32r, tag="drow", bufs=4,
                                name=f"drow{qc}_{h}")
                nc.gpsimd.dma_start(out=drow[64:65, :], in_=drec)
                if pend_norm[0] is not None:
                    pend_norm[0]()
                pend_norm[0] = make_norm(qc, qs, h, otc, drow)

            pend = None
            # last chunk ends on an even head: its normalize writes oT_sb
            # directly (no place-DMA), shortening the tail chain
            horder = (1, 3, 2, 0) if qc == _NQC - 1 else range(_HPG)
            for h in horder:
                c, pb = h // 2, 64 * (h % 2)
                ot = ops.tile([_HD + 1, _QC], f32, tag="ot")
                for g in range(ngr):
                    kts = list(range(g * _GK, min((g + 1) * _GK, nkt)))
                    sp = sps.tile([128, _GK * _QC], f32, tag="sp")
                    # q columns < dj*128 of a diagonal k-tile are entirely
                    # in the causal-masked region: skip them in scores,
                    # exp and AV (triangular decomposition)
                    for i, kt in enumerate(kts):
                        dj = kt - qc * (_QC // 128)
                        lo = max(dj, 0) * 128
                        nc.tensor.matmul(
                            sp[:, i * _QC + lo:(i + 1) * _QC],
                            kT_sb[pb:pb + 64, c, kt * 128:(kt + 1) * 128],
                            qT_sb[pb:pb + 64, c, q0 + lo:q0 + _QC],
                            start=True, stop=(dj < 0),
                            skip_group_check=True)
                    # causal mask: accumulate -1e5 upper-tri into the
                    # diagonal 128-col slice of each diagonal k-tile.
                    # Emitted after BOTH scores: back-to-back with its own
                    # score it pays a ~105ns pipeline restart for the
                    # read-after-write on just-written psum columns
                    for i, kt in enumerate(kts):
                        dj = kt - qc * (_QC // 128)
                        if dj >= 0:
                            lo = dj * 128
                            nc.tensor.matmul(
                                sp[:, i * _QC + lo:i * _QC + lo + 128],
                                mask_sb[:, :], id_sb[:, :],
                                start=False, stop=True,
                                skip_group_check=True)
                    # software pipeline: the previous group's AV runs on
                    # the PE while this group's exp runs on ACT
                    flush_pend(pend)
                    pt = ptp.tile([128, _GK * _QC], bf16, tag="pt")
                    diag = any(kt - qc * (_QC // 128) >= 0 for kt in kts)
                    if not diag:
                        na = len(kts) * _QC
                        nc.scalar.activation(pt[:, :na], sp[:, :na], EXP,
                                             scale=0.125)
                    else:
                        # ragged tile starts: exp per tile's written span
                        for i, kt in enumerate(kts):
                            lo = max(kt - qc * (_QC // 128), 0) * 128
                            nc.scalar.activation(
                                pt[:, i * _QC + lo:(i + 1) * _QC],
                                sp[:, i * _QC + lo:(i + 1) * _QC], EXP,
                                scale=0.125)
                    pend = {"h": h, "kts": kts, "pt": pt, "ot": ot}
                    pop_filler()
            flush_pend(pend)
            pend = None
        # tail: the last head's normalization (whose reciprocal-dance DMAs
        # are still in flight) is sandwiched between independent oproj(2)
        # thunks so its PE broadcast matmul never heads the idle queue
        drain_fillers()
        for t in tail_op1:
            t()
        op2 = oproj_thunks(2, copy_alt=True)
        for t in op2[:5]:
            t()
        pend_norm[0]()
        pend_norm[0] = None
        for t in op2[5:]:
            t()
        for t in oproj_thunks(3, copy_alt=True):
            t()
    nc.compile()
    return nc


def get_nc():
    if "nc" not in _CACHE:
        _CACHE["nc"] = _build_nc()
    return _CACHE["nc"]


def make_in_maps(x, token_positions, Q, K, V, O_w):
    """Host-side sharding: per-core input dict (core = b*4 + hg)."""
    import ml_dtypes
    bf16 = ml_dtypes.bfloat16
    x = np.asarray(x, dtype=np.float32)
    tp = np.asarray(token_positions)
    Q = np.asarray(Q, dtype=np.float32)
    K = np.asarray(K, dtype=np.float32)
    V = np.asarray(V, dtype=np.float32)
    O_w = np.asarray(O_w, dtype=np.float32)

    # RoPE tables, [128, L]: rows = head-local e (cos/sin repeated pairwise),
    # duplicated for the two heads per partition-tile.
    i = np.arange(_HD // 2, dtype=np.float64)
    denom = _THETA ** (2.0 * i / _HD)                      # [32]
    ang = tp.astype(np.float64)[None, :] / denom[:, None]  # [32, L]
    cs2 = np.repeat(np.cos(ang), 2, axis=0).astype(np.float32)
    sn2 = np.repeat(np.sin(ang), 2, axis=0).astype(np.float32)
    cs2 = np.concatenate([cs2, cs2], axis=0)               # [128, L]
    sn2 = np.concatenate([sn2, sn2], axis=0)

    # pairwise-rotation permutation (rot(x)[2i] = -x[2i+1], rot[2i+1] = x[2i])
    # as a stationary operand: out = permT.T @ x^T = Perm @ x^T
    p64 = np.zeros((64, 64), np.float32)
    for j in range(_HD // 2):
        p64[2 * j + 1, 2 * j] = -1.0
        p64[2 * j, 2 * j + 1] = 1.0
    permT = np.zeros((128, 128), np.float32)
    permT[0:64, 0:64] = p64
    permT[64:128, 64:128] = p64

    # causal mask as an additive stationary operand: matmul adds
    # maskM.T (-1e5 where q' < k) into the diagonal score tile
    a = np.arange(128)
    maskM = np.where(a[:, None] < a[None, :], -1.0e5, 0.0).astype(np.float32)
    id128 = np.eye(128, dtype=np.float32)

    Qr = Q.reshape(_H, _HD, _D)
    Kr = K.reshape(_H, _HD, _D)
    Vr = V.reshape(_H, _HD, _D)

    in_maps = []
    xT = [np.ascontiguousarray(x[b].T).astype(bf16) for b in range(_B)]
    for core in range(_NCORES):
        b, hg = core // 4, core % 4
        hs = slice(hg * _HPG, (hg + 1) * _HPG)
        in_maps.append({
            "xT": xT[b],
            "wq": Qr[hs].reshape(_EG, _D).T.astype(bf16),
            "wk": Kr[hs].reshape(_EG, _D).T.astype(bf16),
            "wv": Vr[hs].reshape(_EG, _D).T.astype(bf16),
            "wo": O_w[:, hg * _EG:(hg + 1) * _EG].T.astype(bf16),
            "cs2": cs2.astype(bf16), "sn2": sn2.astype(bf16),
            "perm": permT.astype(bf16),
            "maskM": maskM.astype(bf16), "id128": id128.astype(bf16),
        })
    return in_maps


def run_on_hw(in_maps, trace=False, **kw):
    from concourse.bass_utils import run_bass_kernel_spmd
    nc = get_nc()
    return run_bass_kernel_spmd(nc, in_maps, core_ids=list(range(_NCORES)),
                                trace=trace, **kw)


def kernel(x, token_positions, Q, K, V, O_w):
    in_maps = make_in_maps(x, token_positions, Q, K, V, O_w)
    res = run_on_hw(in_maps)
    out = np.zeros((_B, _L, _D), dtype=np.float32)
    for core in range(_NCORES):
        out[core // 4] += res.results[core]["y"]
    return out
